# revision 14
# baseline (speedup 1.0000x reference)
"""Bresenham (border-ring) attention kernel for Trainium2, 8 NeuronCores.

Computation (per full input):
    att  = einsum('bchw,c->bhw', x, w) + b        # 1x1 conv to 1 channel
    att  = sigmoid(att)
    mask = border ring of the HxW rectangle       # 1 on border, 0 inside
    out  = x * (att * (1 + mask))[:, None]

Pure bandwidth problem: whole pipeline in fp16 halves HBM traffic to
~103 MB/core (~290 us floor at the per-NC HBM limit); rel err ~1e-3 vs
the 2e-2 gate.

Measured engine facts driving this schedule (perfetto traces of 4 earlier
versions):
  - PE runs at the 1.2 GHz p-state.  A pure stream of M=1 contraction
    matmuls runs ~330 ns each; mixing in K=1/M=128 broadcast matmuls
    (128-col LDWEIGHTS) degrades every matmul to ~510-650 ns.  So only
    4 of 7 subtiles broadcast via PE.
  - DVE tensor_tensor and ANY GpSimd op serialize on the shared SBUF
    port pair (exclusive lock), so GpSimd broadcast work must fit with
    DVE work inside the per-block DMA window: GpSimd gets 3 of 7
    subtiles (partition_broadcast, fp16 in/out, no cast needed,
    ~138 GB/s), PE+ACT-cast the other 4.
  - TRN2 matmuls write only f32 PSUM -> the PE-broadcast subtiles need
    a PSUM->SBUF fp16 cast; all 4 go on ACT (it has slack).
  - The border (1+mask) factor costs no full-width multiply: the ring
    columns are 2-wide runs every 224 columns, so a couple of tiny
    in-place strided DVE tensor_scalar muls (x2.0) on the sigmoid row
    apply it exactly.

Per superblock of FD=3584 cols (x2 batch, x14 blocks, ~10 us DMA each):
    PE : 7x2 contraction matmuls [128,1]^T@[128,512] -> [1,512] f32
    ACT: 7 sigmoids [1,512] PSUM -> fp16 row st
    DVE: ~4 tiny in-place x2 ops on st border runs
    POOL: partition_broadcast st[0:1536] -> cmb[:,0:1536]  (~3 us)
    PE : 4 broadcast matmuls ones^T @ st -> psB (subtiles 3..6)
    ACT: 4 casts psB -> cmb fp16
    DVE: 4 multiplies ot[h] = xt[h]*cmb per half ([128,1792] fp16 2x)
    DMA: load 1.79 MB (sync ring), 2 stores 0.92 MB (scalar ring)

Engine budget per core (28 blk-iters): DMA ~270-290 us (bound),
PE ~220 us, DVE+Pool ~245 us serialized, ACT ~205 us.
"""

import numpy as np

import concourse.bacc as bacc
import concourse.bass as bass
import concourse.tile as tile
from concourse import mybir
from concourse.bass_utils import run_bass_kernel_spmd

B, C, H, W = 16, 256, 224, 224
HW = H * W  # 50176
NCORES = 8
BLOC = B // NCORES  # 2

FD = 3584            # superblock free dim (spatial columns per tile)
SUB = 512            # matmul subtile (one PSUM bank of f32)
NSUB = FD // SUB     # 7
NBLK = HW // FD      # 14
HALF = FD // 2       # 1792 (store/mul granularity)

POOL_COLS = 4 * SUB  # subtiles 0-3 broadcast on GpSimd (serial slack measured)
POOL_LAST = POOL_COLS // SUB - 1   # last subtile of the pool range (2)

F16 = mybir.dt.float16
F32 = mybir.dt.float32

# stash of the last BassKernelResults (test.py reads exec_time_ns from here)
LAST_RESULTS = None
_NC_CACHE = {}


def _border_runs(blk, lo, hi):
    """Maximal runs of border columns of superblock `blk` within [lo, hi)."""
    n0 = blk * FD
    cols = []
    for n in range(lo, hi):
        g = n0 + n
        y, xcol = g // W, g % W
        if y == 0 or y == H - 1 or xcol == 0 or xcol == W - 1:
            cols.append(n)
    runs = []
    for c in cols:
        if runs and runs[-1][0] + runs[-1][1] == c:
            runs[-1][1] += 1
        else:
            runs.append([c, 1])
    return [tuple(r) for r in runs]


def _border_ops(blk, lo, hi):
    """Group runs into ops: ('strided', start, n) = n runs of 2 cols spaced W;
    ('contig', start, length) otherwise.  Coverage is asserted exact."""
    runs = _border_runs(blk, lo, hi)
    ops = []
    i = 0
    while i < len(runs):
        s, ln = runs[i]
        if ln == 2:
            n = 1
            while (i + n < len(runs) and runs[i + n][1] == 2
                   and runs[i + n][0] == s + n * W
                   and s + (n + 1) * W <= FD):
                n += 1
            ops.append(("strided", s, n))
            i += n
        else:
            ops.append(("contig", s, ln))
            i += 1
    cover = set()
    for kind, a, n in ops:
        if kind == "strided":
            for k in range(n):
                cover.add(a + k * W)
                cover.add(a + k * W + 1)
        else:
            cover.update(range(a, a + n))
    expect = {c for r in runs for c in range(r[0], r[0] + r[1])}
    assert cover == expect, (blk, lo, hi, ops)
    return ops


def _emit_border(nc, st, blk, lo, hi):
    for kind, a, n in _border_ops(blk, lo, hi):
        if kind == "strided":
            seg = st[:, a:a + n * W].rearrange("p (k q) -> p k q", q=W)
            nc.vector.tensor_scalar_mul(seg[:, :, 0:2], seg[:, :, 0:2], 2.0)
        else:
            nc.vector.tensor_scalar_mul(st[:, a:a + n], st[:, a:a + n], 2.0)


def _build_nc():
    nc = bacc.Bacc("TRN2", debug=False)

    x = nc.dram_tensor("x", [BLOC, C, HW], F16, kind="ExternalInput")
    w0 = nc.dram_tensor("w0", [128, 1], F16, kind="ExternalInput")
    w1 = nc.dram_tensor("w1", [128, 1], F16, kind="ExternalInput")
    ones1 = nc.dram_tensor("ones1", [1, 128], F16, kind="ExternalInput")
    bias1 = nc.dram_tensor("bias1", [1, 1], F32, kind="ExternalInput")
    out = nc.dram_tensor("out", [BLOC, C, HW], F16, kind="ExternalOutput")

    # view [BLOC, C, HW] as [BLOC, p=128, h=2, n]: c = h*128 + p
    x_r = x.ap().rearrange("b (h p) n -> b p h n", h=2)
    out_r = out.ap().rearrange("b (h p) n -> b p h n", h=2)

    with tile.TileContext(nc) as tc:
        with (
            tc.tile_pool(name="consts", bufs=1) as consts,
            tc.tile_pool(name="xin", bufs=6) as xin_pool,
            tc.tile_pool(name="oout", bufs=4) as out_pool,
            tc.tile_pool(name="spool", bufs=2) as s_pool,
            tc.tile_pool(name="cpool", bufs=2) as c_pool,
            tc.tile_pool(name="psA", bufs=4, space="PSUM") as psA,
            tc.tile_pool(name="psB", bufs=3, space="PSUM") as psB,
        ):
            w0_t = consts.tile([128, 1], F16)
            nc.sync.dma_start(out=w0_t[:], in_=w0.ap())
            w1_t = consts.tile([128, 1], F16)
            nc.sync.dma_start(out=w1_t[:], in_=w1.ap())
            ones1_t = consts.tile([1, 128], F16)
            nc.sync.dma_start(out=ones1_t[:], in_=ones1.ap())
            bias1_t = consts.tile([1, 1], F32)
            nc.sync.dma_start(out=bias1_t[:], in_=bias1.ap())

            for blk in range(NBLK):
                n0 = blk * FD
                for b in range(BLOC):
                    xt = xin_pool.tile([128, 2, FD], F16)
                    # two half-loads: subtiles 0-3 only depend on the first
                    # half, shortening the per-iteration dependency chain
                    nc.sync.dma_start(
                        out=xt[:, :, 0:HALF],
                        in_=x_r[b, :, :, n0:n0 + HALF])
                    nc.sync.dma_start(
                        out=xt[:, :, HALF:FD],
                        in_=x_r[b, :, :, n0 + HALF:n0 + FD])
                    ot = out_pool.tile([128, 2, FD], F16)
                    st = s_pool.tile([1, FD], F16)
                    cmb = c_pool.tile([128, FD], F16)

                    for j in range(NSUB):
                        js = slice(j * SUB, (j + 1) * SUB)
                        ps_att = psA.tile([1, SUB], F32)
                        nc.tensor.matmul(
                            ps_att[:], w0_t[:], xt[:, 0, js],
                            start=True, stop=False,
                        )
                        nc.tensor.matmul(
                            ps_att[:], w1_t[:], xt[:, 1, js],
                            start=False, stop=True,
                        )
                        nc.scalar.activation(
                            out=st[:, js],
                            in_=ps_att[:],
                            func=mybir.ActivationFunctionType.Sigmoid,
                            bias=bias1_t[:],
                            scale=1.0,
                        )
                        if j == POOL_LAST:
                            # border x2 then gpsimd broadcast for cols
                            # [0, POOL_COLS) while PE streams subtiles 3..6
                            _emit_border(nc, st, blk, 0, POOL_COLS)
                            nc.gpsimd.partition_broadcast(
                                cmb[:, 0:POOL_COLS], st[:, 0:POOL_COLS])

                    _emit_border(nc, st, blk, POOL_COLS, FD)
                    for j in range(POOL_LAST + 1, NSUB):
                        js = slice(j * SUB, (j + 1) * SUB)
                        ps_bc = psB.tile([128, SUB], F32)
                        nc.tensor.matmul(
                            ps_bc[:], ones1_t[:], st[:, js],
                            start=True, stop=True,
                        )
                        # one cast on DVE (it has the most slack), rest on ACT
                        if j == NSUB - 1:
                            nc.vector.tensor_copy(cmb[:, js], ps_bc[:])
                        else:
                            nc.scalar.copy(cmb[:, js], ps_bc[:])

                    for half in range(2):
                        hs = slice(half * HALF, (half + 1) * HALF)
                        nc.vector.tensor_mul(
                            ot[:, 0, hs], xt[:, 0, hs], cmb[:, hs])
                        nc.vector.tensor_mul(
                            ot[:, 1, hs], xt[:, 1, hs], cmb[:, hs])
                        nc.scalar.dma_start(
                            out=out_r[b, :, :, n0 + half * HALF:
                                      n0 + (half + 1) * HALF],
                            in_=ot[:, :, hs])

    nc.compile()
    return nc


def _host_consts(conv_w, conv_b):
    w = np.asarray(conv_w, dtype=np.float32).reshape(C).astype(np.float16)
    w0 = w[:128, None].copy()                              # [128, 1]
    w1 = w[128:, None].copy()                              # [128, 1]
    ones1 = np.ones((1, 128), dtype=np.float16)            # [1, 128]
    bias1 = np.full((1, 1), np.asarray(conv_b).reshape(-1)[0], dtype=np.float32)
    return dict(w0=w0, w1=w1, ones1=ones1, bias1=bias1)


def kernel(x, conv_w, conv_b):
    global LAST_RESULTS
    x = np.asarray(x)
    assert x.shape == (B, C, H, W), x.shape

    if "nc" not in _NC_CACHE:
        _NC_CACHE["nc"] = _build_nc()
    nc = _NC_CACHE["nc"]

    consts = _host_consts(conv_w, conv_b)
    x_flat = x.reshape(B, C, HW)

    in_maps = []
    for i in range(NCORES):
        xs16 = np.ascontiguousarray(
            x_flat[i * BLOC:(i + 1) * BLOC]).astype(np.float16)
        m = {"x": xs16}
        m.update(consts)
        in_maps.append(m)

    res = run_bass_kernel_spmd(nc, in_maps, list(range(NCORES)))
    LAST_RESULTS = res

    out = np.concatenate(
        [r["out"].reshape(BLOC, C, H, W) for r in res.results], axis=0
    ).astype(np.float32)
    return out


# revision 15
# speedup vs baseline: 1.0720x; 1.0720x over previous
"""Bresenham (border-ring) attention kernel for Trainium2, 8 NeuronCores.

Computation (per full input):
    att  = einsum('bchw,c->bhw', x, w) + b        # 1x1 conv to 1 channel
    att  = sigmoid(att)
    mask = border ring of the HxW rectangle       # 1 on border, 0 inside
    out  = x * (att * (1 + mask))[:, None]

Pure bandwidth problem: whole pipeline in fp16 halves HBM traffic to
~103 MB/core (~290 us floor at the per-NC HBM limit); rel err ~1e-3 vs
the 2e-2 gate.

Measured engine facts driving this schedule (perfetto traces of 4 earlier
versions):
  - PE runs at the 1.2 GHz p-state.  A pure stream of M=1 contraction
    matmuls runs ~330 ns each; mixing in K=1/M=128 broadcast matmuls
    (128-col LDWEIGHTS) degrades every matmul to ~510-650 ns.  So only
    4 of 7 subtiles broadcast via PE.
  - DVE tensor_tensor and ANY GpSimd op serialize on the shared SBUF
    port pair (exclusive lock), so GpSimd broadcast work must fit with
    DVE work inside the per-block DMA window: GpSimd gets 3 of 7
    subtiles (partition_broadcast, fp16 in/out, no cast needed,
    ~138 GB/s), PE+ACT-cast the other 4.
  - TRN2 matmuls write only f32 PSUM -> the PE-broadcast subtiles need
    a PSUM->SBUF fp16 cast; all 4 go on ACT (it has slack).
  - The border (1+mask) factor costs no full-width multiply: the ring
    columns are 2-wide runs every 224 columns, so a couple of tiny
    in-place strided DVE tensor_scalar muls (x2.0) on the sigmoid row
    apply it exactly.

Per superblock of FD=3584 cols (x2 batch, x14 blocks, ~10 us DMA each):
    PE : 7x2 contraction matmuls [128,1]^T@[128,512] -> [1,512] f32
    ACT: 7 sigmoids [1,512] PSUM -> fp16 row st
    DVE: ~4 tiny in-place x2 ops on st border runs
    POOL: partition_broadcast st[0:1536] -> cmb[:,0:1536]  (~3 us)
    PE : 4 broadcast matmuls ones^T @ st -> psB (subtiles 3..6)
    ACT: 4 casts psB -> cmb fp16
    DVE: 4 multiplies ot[h] = xt[h]*cmb per half ([128,1792] fp16 2x)
    DMA: load 1.79 MB (sync ring), 2 stores 0.92 MB (scalar ring)

Engine budget per core (28 blk-iters): DMA ~270-290 us (bound),
PE ~220 us, DVE+Pool ~245 us serialized, ACT ~205 us.
"""

import numpy as np

import concourse.bacc as bacc
import concourse.bass as bass
import concourse.tile as tile
from concourse import mybir
from concourse.bass_utils import run_bass_kernel_spmd

B, C, H, W = 16, 256, 224, 224
HW = H * W  # 50176
NCORES = 8
BLOC = B // NCORES  # 2

FD = 3584            # superblock free dim (spatial columns per tile)
SUB = 512            # matmul subtile (one PSUM bank of f32)
NSUB = FD // SUB     # 7
NBLK = HW // FD      # 14
HALF = FD // 2       # 1792 (store/mul granularity)

POOL_COLS = 3 * SUB  # subtiles 0-2 broadcast on GpSimd
POOL_LAST = POOL_COLS // SUB - 1   # last subtile of the pool range (2)

F16 = mybir.dt.float16
F32 = mybir.dt.float32

# stash of the last BassKernelResults (test.py reads exec_time_ns from here)
LAST_RESULTS = None
_NC_CACHE = {}


def _border_runs(blk, lo, hi):
    """Maximal runs of border columns of superblock `blk` within [lo, hi)."""
    n0 = blk * FD
    cols = []
    for n in range(lo, hi):
        g = n0 + n
        y, xcol = g // W, g % W
        if y == 0 or y == H - 1 or xcol == 0 or xcol == W - 1:
            cols.append(n)
    runs = []
    for c in cols:
        if runs and runs[-1][0] + runs[-1][1] == c:
            runs[-1][1] += 1
        else:
            runs.append([c, 1])
    return [tuple(r) for r in runs]


def _border_ops(blk, lo, hi):
    """Group runs into ops: ('strided', start, n) = n runs of 2 cols spaced W;
    ('contig', start, length) otherwise.  Coverage is asserted exact."""
    runs = _border_runs(blk, lo, hi)
    ops = []
    i = 0
    while i < len(runs):
        s, ln = runs[i]
        if ln == 2:
            n = 1
            while (i + n < len(runs) and runs[i + n][1] == 2
                   and runs[i + n][0] == s + n * W
                   and s + (n + 1) * W <= FD):
                n += 1
            ops.append(("strided", s, n))
            i += n
        else:
            ops.append(("contig", s, ln))
            i += 1
    cover = set()
    for kind, a, n in ops:
        if kind == "strided":
            for k in range(n):
                cover.add(a + k * W)
                cover.add(a + k * W + 1)
        else:
            cover.update(range(a, a + n))
    expect = {c for r in runs for c in range(r[0], r[0] + r[1])}
    assert cover == expect, (blk, lo, hi, ops)
    return ops


def _emit_border(nc, st, blk, lo, hi):
    for kind, a, n in _border_ops(blk, lo, hi):
        if kind == "strided":
            seg = st[:, a:a + n * W].rearrange("p (k q) -> p k q", q=W)
            nc.vector.tensor_scalar_mul(seg[:, :, 0:2], seg[:, :, 0:2], 2.0)
        else:
            nc.vector.tensor_scalar_mul(st[:, a:a + n], st[:, a:a + n], 2.0)


def _build_nc():
    nc = bacc.Bacc("TRN2", debug=False)

    x = nc.dram_tensor("x", [BLOC, C, HW], F16, kind="ExternalInput")
    w0 = nc.dram_tensor("w0", [128, 1], F16, kind="ExternalInput")
    w1 = nc.dram_tensor("w1", [128, 1], F16, kind="ExternalInput")
    ones1 = nc.dram_tensor("ones1", [1, 128], F16, kind="ExternalInput")
    bias1 = nc.dram_tensor("bias1", [1, 1], F32, kind="ExternalInput")
    out = nc.dram_tensor("out", [BLOC, C, HW], F16, kind="ExternalOutput")

    # view [BLOC, C, HW] as [BLOC, p=128, h=2, n]: c = h*128 + p
    x_r = x.ap().rearrange("b (h p) n -> b p h n", h=2)
    out_r = out.ap().rearrange("b (h p) n -> b p h n", h=2)

    with tile.TileContext(nc) as tc:
        with (
            tc.tile_pool(name="consts", bufs=1) as consts,
            tc.tile_pool(name="xin", bufs=6) as xin_pool,
            tc.tile_pool(name="oout", bufs=4) as out_pool,
            tc.tile_pool(name="spool", bufs=2) as s_pool,
            tc.tile_pool(name="cpool", bufs=2) as c_pool,
            tc.tile_pool(name="psA", bufs=3, space="PSUM") as psA,
            tc.tile_pool(name="psB", bufs=4, space="PSUM") as psB,
        ):
            w0_t = consts.tile([128, 1], F16)
            nc.sync.dma_start(out=w0_t[:], in_=w0.ap())
            w1_t = consts.tile([128, 1], F16)
            nc.sync.dma_start(out=w1_t[:], in_=w1.ap())
            ones1_t = consts.tile([1, 128], F16)
            nc.sync.dma_start(out=ones1_t[:], in_=ones1.ap())
            bias1_t = consts.tile([1, 1], F32)
            nc.sync.dma_start(out=bias1_t[:], in_=bias1.ap())

            for blk in range(NBLK):
                n0 = blk * FD
                for b in range(BLOC):
                    xt = xin_pool.tile([128, 2, FD], F16)
                    # two half-loads: subtiles 0-3 only depend on the first
                    # half, shortening the per-iteration dependency chain
                    nc.sync.dma_start(
                        out=xt[:, :, 0:HALF],
                        in_=x_r[b, :, :, n0:n0 + HALF])
                    nc.sync.dma_start(
                        out=xt[:, :, HALF:FD],
                        in_=x_r[b, :, :, n0 + HALF:n0 + FD])
                    ot = out_pool.tile([128, 2, FD], F16)
                    st = s_pool.tile([1, FD], F16)
                    cmb = c_pool.tile([128, FD], F16)

                    for j in range(NSUB):
                        js = slice(j * SUB, (j + 1) * SUB)
                        ps_att = psA.tile([1, SUB], F32)
                        nc.tensor.matmul(
                            ps_att[:], w0_t[:], xt[:, 0, js],
                            start=True, stop=False,
                        )
                        nc.tensor.matmul(
                            ps_att[:], w1_t[:], xt[:, 1, js],
                            start=False, stop=True,
                        )
                        nc.scalar.activation(
                            out=st[:, js],
                            in_=ps_att[:],
                            func=mybir.ActivationFunctionType.Sigmoid,
                            bias=bias1_t[:],
                            scale=1.0,
                        )
                        if j == POOL_LAST:
                            # border x2 then gpsimd broadcast for cols
                            # [0, POOL_COLS) while PE streams subtiles 3..6
                            _emit_border(nc, st, blk, 0, POOL_COLS)
                            nc.gpsimd.partition_broadcast(
                                cmb[:, 0:POOL_COLS], st[:, 0:POOL_COLS])

                    _emit_border(nc, st, blk, POOL_COLS, FD)
                    for j in range(POOL_LAST + 1, NSUB):
                        js = slice(j * SUB, (j + 1) * SUB)
                        ps_bc = psB.tile([128, SUB], F32)
                        nc.tensor.matmul(
                            ps_bc[:], ones1_t[:], st[:, js],
                            start=True, stop=True,
                        )
                        # one cast on DVE (it has the most slack), rest on ACT
                        if j == NSUB - 1:
                            nc.vector.tensor_copy(cmb[:, js], ps_bc[:])
                        else:
                            nc.scalar.copy(cmb[:, js], ps_bc[:])

                    for half in range(2):
                        hs = slice(half * HALF, (half + 1) * HALF)
                        nc.vector.tensor_mul(
                            ot[:, 0, hs], xt[:, 0, hs], cmb[:, hs])
                        nc.vector.tensor_mul(
                            ot[:, 1, hs], xt[:, 1, hs], cmb[:, hs])
                        nc.scalar.dma_start(
                            out=out_r[b, :, :, n0 + half * HALF:
                                      n0 + (half + 1) * HALF],
                            in_=ot[:, :, hs])

    nc.compile()
    return nc


def _host_consts(conv_w, conv_b):
    w = np.asarray(conv_w, dtype=np.float32).reshape(C).astype(np.float16)
    w0 = w[:128, None].copy()                              # [128, 1]
    w1 = w[128:, None].copy()                              # [128, 1]
    ones1 = np.ones((1, 128), dtype=np.float16)            # [1, 128]
    bias1 = np.full((1, 1), np.asarray(conv_b).reshape(-1)[0], dtype=np.float32)
    return dict(w0=w0, w1=w1, ones1=ones1, bias1=bias1)


def kernel(x, conv_w, conv_b):
    global LAST_RESULTS
    x = np.asarray(x)
    assert x.shape == (B, C, H, W), x.shape

    if "nc" not in _NC_CACHE:
        _NC_CACHE["nc"] = _build_nc()
    nc = _NC_CACHE["nc"]

    consts = _host_consts(conv_w, conv_b)
    x_flat = x.reshape(B, C, HW)

    in_maps = []
    for i in range(NCORES):
        xs16 = np.ascontiguousarray(
            x_flat[i * BLOC:(i + 1) * BLOC]).astype(np.float16)
        m = {"x": xs16}
        m.update(consts)
        in_maps.append(m)

    res = run_bass_kernel_spmd(nc, in_maps, list(range(NCORES)))
    LAST_RESULTS = res

    out = np.concatenate(
        [r["out"].reshape(BLOC, C, H, W) for r in res.results], axis=0
    ).astype(np.float32)
    return out
